# revision 17
# baseline (speedup 1.0000x reference)
"""Trainium2 Bass kernel for CodeRecursiveNeuralNetworks (tree-RNN over complete
binary trees, heap layout).

Math (per tree, heap order: node i has parent (i-1)//2, level d = [2^d-1, 2^{d+1}-1)):
    x = E[node_type];  h_leaf = tanh(x_leaf)
    for d = 8..0:  h_d = tanh(x_d + (h_{d+1,even} + h_{d+1,odd}) @ Wh + bh)
    logits = h_root @ Wc + bc;  out = log_softmax(logits)

Strategy (8 cores, data-parallel over trees; 32 trees/core, no collectives):
  - Everything in "transposed" layout [H=128 partitions, nodes free]; with levels
    stored tree-major, children of parent column p are exactly columns 2p, 2p+1.
  - Embedding lookups become one-hot matmuls on the TensorEngine (VOCAB=100 <= 128):
    the host re-encodes node_type as fp8e4 one-hot columns (index re-encoding only),
    streamed as a few big DMA "bands" over both HWDGE rings.
  - The leaf level is folded away: G = tanh(E) @ Wh is precomputed on device, and
    level-8 gets  psum8 = E^T @ onehot8 + G^T @ C  where C[:,p] = onehot(a_p)+onehot(b_p)
    sums the two leaf children of parent p (so leaves never materialize).
  - Levels 7..6 (big): pairsum on VectorE (stride-2 add, fp16 out) + one fp16 Wh
    matmul. Levels 5..0 (latency-bound): pairsum folded into the PE as two stride-2
    fp16 matmuls, h stored fp16 — no VectorE hop in the serial chain.
  - tanh+bias fused on ScalarE reading PSUM directly; h for levels 8..7 kept fp32.
  - A 26-matmul warm-up burst opens the PE HAM clock gate (1.2 -> 2.4 GHz) while
    the first one-hot bands are still in flight.
  - Final log_softmax on device; per-core output [32, 6] fp32, concatenated on host.
  - Measured: ~49 us HW exec, rel err ~8e-4 vs fp32 reference.
"""

import numpy as np
import ml_dtypes

B = 256
M = 1023
H = 128
V = 100
NCLS = 6
MAX_DEPTH = 10
CORES = 8
TPC = B // CORES          # trees per core
N8 = TPC * 256            # level-8 nodes per core (8192)

# one-hot column block offsets within the single "oh" input tensor
#   [ C8 (8192) | x8 (8192) | x7 (4096) | ... | x0 (32) ]
_LEVEL_N = {d: TPC * (1 << d) for d in range(MAX_DEPTH)}   # cols per level, per core
OFF_C8 = 0
_off = N8
OFF_X = {}
for _d in range(8, -1, -1):
    OFF_X[_d] = _off
    _off += _LEVEL_N[_d]
TOTAL_COLS = _off          # 24544

BF16 = ml_dtypes.bfloat16

_PROGRAM = None            # cached (nc, out_name)


def _build_program():
    import concourse.bacc as bacc
    import concourse.tile as tile
    import concourse.mybir as mybir
    from concourse.masks import make_identity

    dt = mybir.dt
    AF = mybir.ActivationFunctionType
    ALU = mybir.AluOpType
    AX = mybir.AxisListType

    nc = bacc.Bacc("TRN2", target_bir_lowering=False, debug=False)

    oh_d = nc.dram_tensor("oh", [V, TOTAL_COLS], dt.float8e4, kind="ExternalInput")
    ebf_d = nc.dram_tensor("e_bf", [V, H], dt.float16, kind="ExternalInput")
    et_d = nc.dram_tensor("e_t", [H, V], dt.float32, kind="ExternalInput")
    whb_d = nc.dram_tensor("wh_b", [H, H], dt.float16, kind="ExternalInput")
    bh_d = nc.dram_tensor("bh", [H, 1], dt.float32, kind="ExternalInput")
    wc_d = nc.dram_tensor("wc", [H, NCLS], dt.float32, kind="ExternalInput")
    bc_d = nc.dram_tensor("bc", [NCLS, 1], dt.float32, kind="ExternalInput")
    out_d = nc.dram_tensor("out", [TPC, NCLS], dt.float32, kind="ExternalOutput")

    with tile.TileContext(nc) as tc:
        with (
            tc.tile_pool(name="const", bufs=1) as cpool,
            tc.tile_pool(name="bandp", bufs=12) as bandpool,
            tc.tile_pool(name="hp", bufs=1) as hpool,
            tc.tile_pool(name="aggp", bufs=3) as aggpool,
            tc.tile_pool(name="psp", bufs=3, space="PSUM") as pspool,
            tc.tile_pool(name="minips", bufs=2, space="PSUM") as minips,
            tc.tile_pool(name="smallp", bufs=1) as smpool,
        ):
            # ---- PE warm-up first: junk memset is the only dependency ----
            junk = cpool.tile([H, 512], dt.bfloat16, tag="junk")
            nc.gpsimd.memset(junk[:], 0)
            warm_ps = minips.tile([H, 512], dt.float32, tag="mini", name="warm_ps")
            for k in range(30):
                nc.tensor.matmul(warm_ps[:], lhsT=junk[:, :H], rhs=junk[:],
                                 start=True, stop=True)

            # ---- constants ----
            e_bf = cpool.tile([V, H], dt.float16, tag="e_bf")
            nc.gpsimd.dma_start(out=e_bf[:], in_=ebf_d[:])
            et = cpool.tile([H, V], dt.float32, tag="et")
            nc.gpsimd.dma_start(out=et[:], in_=et_d[:])
            whb = cpool.tile([H, H], dt.float16, tag="whb")
            nc.gpsimd.dma_start(out=whb[:], in_=whb_d[:])
            bh_t = cpool.tile([H, 1], dt.float32, tag="bh")
            nc.gpsimd.dma_start(out=bh_t[:], in_=bh_d[:])
            wc_t = cpool.tile([H, NCLS], dt.float32, tag="wc")
            nc.gpsimd.dma_start(out=wc_t[:], in_=wc_d[:])
            bc_t = cpool.tile([NCLS, 1], dt.float32, tag="bc")
            nc.gpsimd.dma_start(out=bc_t[:], in_=bc_d[:])
            id6 = cpool.tile([NCLS, NCLS], dt.float32, tag="id6")
            make_identity(nc, id6[:])
            wc16 = cpool.tile([H, NCLS], dt.float16, tag="wc16")
            nc.vector.tensor_copy(wc16[:], wc_t[:])

            h_tiles = {d: [] for d in range(9)}

            # ---- one-hot bands: few big DMAs instead of many small ones ----
            BAND = 2048
            bands = {}
            _band_eng = [0]

            def load_band(col0, ncols, nm):
                t = bandpool.tile([V, BAND], dt.float8e4, tag="band", name=nm)
                eng = nc.sync if _band_eng[0] % 2 == 0 else nc.scalar
                _band_eng[0] += 1
                eng.dma_start(out=t[:, :ncols], in_=oh_d[:, col0:col0 + ncols])
                return t

            def band_slice(kind, d, gcol, w):
                base = OFF_C8 if kind == "c" else OFF_X[d]
                bi = gcol // BAND
                off = gcol % BAND
                return bands[(kind, d, bi)][:, off:off + w]

            # preload every band up-front (c8/x8 first), alternating rings.
            band_order = [("c", 8), ("x", 8), ("x", 7), ("x", 6), ("x", 5),
                          ("x", 4), ("x", 3), ("x", 2), ("x", 1), ("x", 0)]
            band_reqs = []
            for kind, d in band_order:
                base = OFF_C8 if kind == "c" else OFF_X[d]
                n = N8 if kind == "c" else _LEVEL_N[d]
                for bi in range((n + BAND - 1) // BAND):
                    band_reqs.append((kind, d, bi, base, min(BAND, n - bi * BAND)))
            # first 4 bands now (2 per ring)...
            for kind, d, bi, base, ncols in band_reqs[:4]:
                bands[(kind, d, bi)] = load_band(base + bi * BAND, ncols,
                                                 f"band_{kind}{d}_{bi}")

            # ---- G = tanh(E) @ Wh  (as lhsT [V, H] bf16) ----
            tanh_et = cpool.tile([H, V], dt.float16, tag="tanh_et")
            nc.scalar.activation(tanh_et[:], et[:], AF.Tanh)
            g_ps = minips.tile([V, H], dt.float32, tag="mini")
            nc.tensor.matmul(g_ps[:], lhsT=tanh_et[:], rhs=whb[:], start=True, stop=True)
            g_sb = cpool.tile([V, H], dt.float16, tag="g_sb")
            nc.vector.tensor_copy(g_sb[:], g_ps[:])

            # ...remaining bands after the tanh is queued on ACT
            for kind, d, bi, base, ncols in band_reqs[4:]:
                bands[(kind, d, bi)] = load_band(base + bi * BAND, ncols,
                                                 f"band_{kind}{d}_{bi}")

            # ---- levels 8..0 ----
            for d in range(8, -1, -1):
                n = _LEVEL_N[d]
                h_dt = dt.float32 if d >= 7 else dt.float16
                pe_pairsum = d <= 5
                PSF = 1024 if d >= 7 else 512
                for pt in range(0, n, PSF):
                    PT = min(PSF, n - pt)
                    ps = pspool.tile([H, 1024], dt.float32, tag="ps",
                                     name=f"ps{d}_{pt}")
                    if d < 8:
                        hc = h_tiles[d + 1][(2 * pt) // 2048]
                        off = (2 * pt) % 2048
                        pairs = hc[:, off:off + 2 * PT].rearrange(
                            "p (n two) -> p n two", two=2)
                        if not pe_pairsum:
                            agg = aggpool.tile([H, 1024], dt.float16, tag="agg",
                                               name=f"agg{d}_{pt}")
                            nc.vector.tensor_tensor(
                                out=agg[:, :PT].rearrange("p (n one) -> p n one",
                                                          one=1),
                                in0=pairs[:, :, 0:1],
                                in1=pairs[:, :, 1:2],
                                op=ALU.add,
                            )
                    subs = [(ps[:, sub:sub + min(512, PT - sub)], pt + sub, sub,
                             min(512, PT - sub)) for sub in range(0, PT, 512)]
                    # two-pass emission: consecutive matmuls hit different psum
                    # slices so fill/drain overlap (same-slice accumulating
                    # pairs serialize on the PE)
                    for sl, gcol, sub, w in subs:
                        nc.tensor.matmul(sl, lhsT=e_bf[:],
                                         rhs=band_slice("x", d, gcol, w),
                                         start=True, stop=False)
                    if d == 8:
                        for sl, gcol, sub, w in subs:
                            nc.tensor.matmul(sl, lhsT=g_sb[:],
                                             rhs=band_slice("c", d, gcol, w),
                                             start=False, stop=True)
                    elif pe_pairsum:
                        for sl, gcol, sub, w in subs:
                            nc.tensor.matmul(sl, lhsT=whb[:],
                                             rhs=pairs[:, sub:sub + w, 0:1],
                                             start=False, stop=False)
                        for sl, gcol, sub, w in subs:
                            nc.tensor.matmul(sl, lhsT=whb[:],
                                             rhs=pairs[:, sub:sub + w, 1:2],
                                             start=False, stop=True)
                    else:
                        for sl, gcol, sub, w in subs:
                            nc.tensor.matmul(sl, lhsT=whb[:],
                                             rhs=agg[:, sub:sub + w],
                                             start=False, stop=True)
                    if pt % 2048 == 0:
                        ht = hpool.tile([H, min(2048, n)], h_dt,
                                        tag=f"h{d}_{pt}", name=f"h{d}_{pt}")
                        h_tiles[d].append(ht)
                    ht = h_tiles[d][pt // 2048]
                    last_tanh = nc.scalar.activation(
                        ht[:, pt % 2048:pt % 2048 + PT],
                        ps[:, :PT], AF.Tanh, bias=bh_t[:])
                if d <= 5:
                    # keep the PE HAM clock-gate open while waiting on tanh
                    for _f in range(2):
                        nc.tensor.matmul(warm_ps[:], lhsT=junk[:, :H], rhs=junk[:],
                                         start=True, stop=True)

            # hoist the Ln table-set switch off the softmax critical path
            from concourse.tile_rust import add_dep_helper
            lnwarm = smpool.tile([1, 1], dt.float32, tag="lnwarm")
            lnw_i = nc.scalar.activation(lnwarm[:], id6[0:1, 0:1], AF.Ln)
            add_dep_helper(last_tanh.ins, lnw_i.ins, sync=True,
                           reason="ln table switch after last tanh")

            # ---- logits + log_softmax ----
            h0 = h_tiles[0][0]                      # [H, TPC]
            lg_ps = minips.tile([NCLS, TPC], dt.float32, tag="mini", name="lg_ps")
            nc.tensor.matmul(lg_ps[:], lhsT=wc16[:],
                             rhs=h0[:], start=True, stop=True)
            lgT = smpool.tile([NCLS, TPC], dt.float32, tag="lgT")
            nc.vector.tensor_tensor(out=lgT[:], in0=lg_ps[:],
                                    in1=bc_t[:].to_broadcast([NCLS, TPC]), op=ALU.add)
            tr_ps = minips.tile([TPC, NCLS], dt.float32, tag="mini", name="tr_ps")
            nc.tensor.transpose(tr_ps[:], lgT[:], id6[:])
            lg = smpool.tile([TPC, NCLS], dt.float32, tag="lg")
            nc.vector.tensor_copy(lg[:], tr_ps[:])
            ex = smpool.tile([TPC, NCLS], dt.float32, tag="ex")
            nc.scalar.activation(ex[:], lg[:], AF.Exp)
            s = smpool.tile([TPC, 1], dt.float32, tag="s")
            nc.vector.tensor_reduce(s[:], ex[:], axis=AX.X, op=ALU.add)
            lns = smpool.tile([TPC, 1], dt.float32, tag="lns")
            nc.scalar.activation(lns[:], s[:], AF.Ln)
            res = smpool.tile([TPC, NCLS], dt.float32, tag="res")
            nc.vector.tensor_scalar(out=res[:], in0=lg[:], scalar1=lns[:], scalar2=None,
                                    op0=ALU.subtract)
            nc.sync.dma_start(out=out_d[:], in_=res[:])

    nc.compile()
    return nc, "out"


def _get_program():
    global _PROGRAM
    if _PROGRAM is None:
        _PROGRAM = _build_program()
    return _PROGRAM


def _host_inputs(node_type, E, Wh, bh, Wc, bc):
    """Build per-core input maps (host side: sharding + index re-encoding)."""
    nt = np.asarray(node_type).astype(np.int64).reshape(B, M)
    E = np.asarray(E, dtype=np.float32)
    Wh = np.asarray(Wh, dtype=np.float32)
    bh = np.asarray(bh, dtype=np.float32)
    Wc = np.asarray(Wc, dtype=np.float32)
    bc = np.asarray(bc, dtype=np.float32)

    shared = {
        "e_bf": E.astype(np.float16),
        "e_t": np.ascontiguousarray(E.T),
        "wh_b": Wh.astype(np.float16),
        "bh": bh.reshape(H, 1),
        "wc": Wc,
        "bc": bc.reshape(NCLS, 1),
    }

    in_maps = []
    for c in range(CORES):
        ntc = nt[c * TPC:(c + 1) * TPC]                  # [32, 1023]
        ohf = np.zeros((V, TOTAL_COLS), dtype=np.float32)
        # C8: summed one-hots of the two leaf children of each level-8 parent
        lt = ntc[:, 511:1023]                            # [32, 512] leaves
        a = lt[:, 0::2].ravel()
        b = lt[:, 1::2].ravel()
        cols = np.arange(N8)
        np.add.at(ohf, (a, OFF_C8 + cols), 1.0)
        np.add.at(ohf, (b, OFF_C8 + cols), 1.0)
        # per-level x one-hots
        for d in range(8, -1, -1):
            idx = ntc[:, (1 << d) - 1:(2 << d) - 1].ravel()
            ohf[idx, OFF_X[d] + np.arange(idx.size)] = 1.0
        in_maps.append({"oh": ohf.astype(ml_dtypes.float8_e4m3), **shared})
    return in_maps


def kernel(node_type, parent_idx, depth, root_idx, E, Wh, bh, Wc, bc,
           _trace=False, _sim=False):
    from concourse.bass_utils import run_bass_kernel_spmd

    nc, out_name = _get_program()
    in_maps = _host_inputs(node_type, E, Wh, bh, Wc, bc)

    if _sim:
        from concourse.bass_interp import CoreSim
        outs = []
        for m in in_maps[:_sim if isinstance(_sim, int) and _sim > 1 else CORES]:
            sim = CoreSim(nc, trace=False)
            for k, v in m.items():
                sim.tensor(k)[:] = v
            sim.simulate(check_with_hw=False)
            outs.append(np.array(sim.tensor(out_name)))
        return np.concatenate(outs, axis=0).astype(np.float32)

    results = run_bass_kernel_spmd(
        nc, in_maps, core_ids=list(range(CORES)), trace=_trace,
    )
    out = np.concatenate([r[out_name] for r in results.results], axis=0)
    out = np.ascontiguousarray(out).astype(np.float32)
    if _trace:
        return out, results
    return out



# revision 18
# speedup vs baseline: 1.0304x; 1.0304x over previous
"""Trainium2 Bass kernel for CodeRecursiveNeuralNetworks (tree-RNN over complete
binary trees, heap layout).

Math (per tree, heap order: node i has parent (i-1)//2, level d = [2^d-1, 2^{d+1}-1)):
    x = E[node_type];  h_leaf = tanh(x_leaf)
    for d = 8..0:  h_d = tanh(x_d + (h_{d+1,even} + h_{d+1,odd}) @ Wh + bh)
    logits = h_root @ Wc + bc;  out = log_softmax(logits)

Strategy (8 cores, data-parallel over trees; 32 trees/core, no collectives):
  - Everything in "transposed" layout [H=128 partitions, nodes free]; with levels
    stored tree-major, children of parent column p are exactly columns 2p, 2p+1.
  - Embedding lookups become one-hot matmuls on the TensorEngine (VOCAB=100 <= 128):
    the host re-encodes node_type as fp8e4 one-hot columns (index re-encoding only),
    streamed as a few big DMA "bands" over both HWDGE rings.
  - The leaf level is folded away: G = tanh(E) @ Wh is precomputed on device, and
    level-8 gets  psum8 = E^T @ onehot8 + G^T @ C  where C[:,p] = onehot(a_p)+onehot(b_p)
    sums the two leaf children of parent p (so leaves never materialize).
  - Levels 7..6 (big): pairsum on VectorE (stride-2 add, fp16 out) + one fp16 Wh
    matmul. Levels 5..0 (latency-bound): pairsum folded into the PE as two stride-2
    fp16 matmuls, h stored fp16 — no VectorE hop in the serial chain.
  - tanh+bias fused on ScalarE reading PSUM directly; h for levels 8..7 kept fp32.
  - A 26-matmul warm-up burst opens the PE HAM clock gate (1.2 -> 2.4 GHz) while
    the first one-hot bands are still in flight.
  - Final log_softmax on device; per-core output [32, 6] fp32, concatenated on host.
  - Tail levels keep the HAM clock-gate open with junk filler matmuls; matmul
    emission is two-pass per psum tile so accumulating pairs pipeline.
  - Measured: ~47 us HW exec, rel err ~8e-4 vs fp32 reference.
"""

import numpy as np
import ml_dtypes

B = 256
M = 1023
H = 128
V = 100
NCLS = 6
MAX_DEPTH = 10
CORES = 8
TPC = B // CORES          # trees per core
N8 = TPC * 256            # level-8 nodes per core (8192)

# one-hot column block offsets within the single "oh" input tensor
#   [ C8 (8192) | x8 (8192) | x7 (4096) | ... | x0 (32) ]
_LEVEL_N = {d: TPC * (1 << d) for d in range(MAX_DEPTH)}   # cols per level, per core
OFF_C8 = 0
_off = N8
OFF_X = {}
for _d in range(8, -1, -1):
    OFF_X[_d] = _off
    _off += _LEVEL_N[_d]
TOTAL_COLS = _off          # 24544

BF16 = ml_dtypes.bfloat16

_PROGRAM = None            # cached (nc, out_name)


def _build_program():
    import concourse.bacc as bacc
    import concourse.tile as tile
    import concourse.mybir as mybir
    from concourse.masks import make_identity

    dt = mybir.dt
    AF = mybir.ActivationFunctionType
    ALU = mybir.AluOpType
    AX = mybir.AxisListType

    nc = bacc.Bacc("TRN2", target_bir_lowering=False, debug=False)

    oh_d = nc.dram_tensor("oh", [V, TOTAL_COLS], dt.float8e4, kind="ExternalInput")
    ebf_d = nc.dram_tensor("e_bf", [V, H], dt.float16, kind="ExternalInput")
    et_d = nc.dram_tensor("e_t", [H, V], dt.float32, kind="ExternalInput")
    whb_d = nc.dram_tensor("wh_b", [H, H], dt.float16, kind="ExternalInput")
    bh_d = nc.dram_tensor("bh", [H, 1], dt.float32, kind="ExternalInput")
    wc_d = nc.dram_tensor("wc", [H, NCLS], dt.float32, kind="ExternalInput")
    bc_d = nc.dram_tensor("bc", [NCLS, 1], dt.float32, kind="ExternalInput")
    out_d = nc.dram_tensor("out", [TPC, NCLS], dt.float32, kind="ExternalOutput")

    with tile.TileContext(nc) as tc:
        with (
            tc.tile_pool(name="const", bufs=1) as cpool,
            tc.tile_pool(name="bandp", bufs=12) as bandpool,
            tc.tile_pool(name="hp", bufs=1) as hpool,
            tc.tile_pool(name="aggp", bufs=3) as aggpool,
            tc.tile_pool(name="psp", bufs=3, space="PSUM") as pspool,
            tc.tile_pool(name="minips", bufs=2, space="PSUM") as minips,
            tc.tile_pool(name="smallp", bufs=1) as smpool,
        ):
            # ---- PE warm-up first: junk memset is the only dependency ----
            junk = cpool.tile([H, 512], dt.bfloat16, tag="junk")
            nc.gpsimd.memset(junk[:], 0)
            warm_ps = minips.tile([H, 512], dt.float32, tag="mini", name="warm_ps")
            for k in range(30):
                nc.tensor.matmul(warm_ps[:], lhsT=junk[:, :H], rhs=junk[:],
                                 start=True, stop=True)

            # ---- constants ----
            e_bf = cpool.tile([V, H], dt.float16, tag="e_bf")
            nc.gpsimd.dma_start(out=e_bf[:], in_=ebf_d[:])
            et = cpool.tile([H, V], dt.float32, tag="et")
            nc.gpsimd.dma_start(out=et[:], in_=et_d[:])
            whb = cpool.tile([H, H], dt.float16, tag="whb")
            nc.gpsimd.dma_start(out=whb[:], in_=whb_d[:])
            bh_t = cpool.tile([H, 1], dt.float32, tag="bh")
            nc.gpsimd.dma_start(out=bh_t[:], in_=bh_d[:])
            wc_t = cpool.tile([H, NCLS], dt.float32, tag="wc")
            nc.gpsimd.dma_start(out=wc_t[:], in_=wc_d[:])
            bc_t = cpool.tile([NCLS, 1], dt.float32, tag="bc")
            nc.gpsimd.dma_start(out=bc_t[:], in_=bc_d[:])
            id6 = cpool.tile([NCLS, NCLS], dt.float32, tag="id6")
            make_identity(nc, id6[:])
            wc16 = cpool.tile([H, NCLS], dt.float16, tag="wc16")
            nc.vector.tensor_copy(wc16[:], wc_t[:])

            h_tiles = {d: [] for d in range(9)}

            # ---- one-hot bands: few big DMAs instead of many small ones ----
            BAND = 2048
            bands = {}
            _band_eng = [0]

            def load_band(col0, ncols, nm):
                t = bandpool.tile([V, BAND], dt.float8e4, tag="band", name=nm)
                eng = nc.sync if _band_eng[0] % 2 == 0 else nc.scalar
                _band_eng[0] += 1
                eng.dma_start(out=t[:, :ncols], in_=oh_d[:, col0:col0 + ncols])
                return t

            def band_slice(kind, d, gcol, w):
                base = OFF_C8 if kind == "c" else OFF_X[d]
                bi = gcol // BAND
                off = gcol % BAND
                return bands[(kind, d, bi)][:, off:off + w]

            # preload every band up-front (c8/x8 first), alternating rings.
            band_order = [("c", 8), ("x", 8), ("x", 7), ("x", 6), ("x", 5),
                          ("x", 4), ("x", 3), ("x", 2), ("x", 1), ("x", 0)]
            band_reqs = []
            for kind, d in band_order:
                base = OFF_C8 if kind == "c" else OFF_X[d]
                n = N8 if kind == "c" else _LEVEL_N[d]
                for bi in range((n + BAND - 1) // BAND):
                    band_reqs.append((kind, d, bi, base, min(BAND, n - bi * BAND)))
            # first 4 bands now (2 per ring)...
            for kind, d, bi, base, ncols in band_reqs[:4]:
                bands[(kind, d, bi)] = load_band(base + bi * BAND, ncols,
                                                 f"band_{kind}{d}_{bi}")

            # ---- G = tanh(E) @ Wh  (as lhsT [V, H] bf16) ----
            tanh_et = cpool.tile([H, V], dt.float16, tag="tanh_et")
            nc.scalar.activation(tanh_et[:], et[:], AF.Tanh)
            g_ps = minips.tile([V, H], dt.float32, tag="mini")
            nc.tensor.matmul(g_ps[:], lhsT=tanh_et[:], rhs=whb[:], start=True, stop=True)
            g_sb = cpool.tile([V, H], dt.float16, tag="g_sb")
            nc.vector.tensor_copy(g_sb[:], g_ps[:])

            # ...remaining bands after the tanh is queued on ACT
            for kind, d, bi, base, ncols in band_reqs[4:]:
                bands[(kind, d, bi)] = load_band(base + bi * BAND, ncols,
                                                 f"band_{kind}{d}_{bi}")

            # ---- levels 8..0 ----
            for d in range(8, -1, -1):
                n = _LEVEL_N[d]
                h_dt = dt.float32 if d >= 7 else dt.float16
                pe_pairsum = d <= 5
                PSF = 1024 if d >= 7 else 512
                for pt in range(0, n, PSF):
                    PT = min(PSF, n - pt)
                    ps = pspool.tile([H, 1024], dt.float32, tag="ps",
                                     name=f"ps{d}_{pt}")
                    if d < 8:
                        hc = h_tiles[d + 1][(2 * pt) // 2048]
                        off = (2 * pt) % 2048
                        pairs = hc[:, off:off + 2 * PT].rearrange(
                            "p (n two) -> p n two", two=2)
                        if not pe_pairsum:
                            agg = aggpool.tile([H, 1024], dt.float16, tag="agg",
                                               name=f"agg{d}_{pt}")
                            nc.vector.tensor_tensor(
                                out=agg[:, :PT].rearrange("p (n one) -> p n one",
                                                          one=1),
                                in0=pairs[:, :, 0:1],
                                in1=pairs[:, :, 1:2],
                                op=ALU.add,
                            )
                    subs = [(ps[:, sub:sub + min(512, PT - sub)], pt + sub, sub,
                             min(512, PT - sub)) for sub in range(0, PT, 512)]
                    # two-pass emission: consecutive matmuls hit different psum
                    # slices so fill/drain overlap (same-slice accumulating
                    # pairs serialize on the PE)
                    for sl, gcol, sub, w in subs:
                        nc.tensor.matmul(sl, lhsT=e_bf[:],
                                         rhs=band_slice("x", d, gcol, w),
                                         start=True, stop=False)
                    if d == 8:
                        for sl, gcol, sub, w in subs:
                            nc.tensor.matmul(sl, lhsT=g_sb[:],
                                             rhs=band_slice("c", d, gcol, w),
                                             start=False, stop=True)
                    elif pe_pairsum:
                        for sl, gcol, sub, w in subs:
                            nc.tensor.matmul(sl, lhsT=whb[:],
                                             rhs=pairs[:, sub:sub + w, 0:1],
                                             start=False, stop=False)
                        for sl, gcol, sub, w in subs:
                            nc.tensor.matmul(sl, lhsT=whb[:],
                                             rhs=pairs[:, sub:sub + w, 1:2],
                                             start=False, stop=True)
                    else:
                        for sl, gcol, sub, w in subs:
                            nc.tensor.matmul(sl, lhsT=whb[:],
                                             rhs=agg[:, sub:sub + w],
                                             start=False, stop=True)
                    if pt % 2048 == 0:
                        ht = hpool.tile([H, min(2048, n)], h_dt,
                                        tag=f"h{d}_{pt}", name=f"h{d}_{pt}")
                        h_tiles[d].append(ht)
                    ht = h_tiles[d][pt // 2048]
                    nc.scalar.activation(ht[:, pt % 2048:pt % 2048 + PT],
                                         ps[:, :PT], AF.Tanh, bias=bh_t[:])
                if d <= 5:
                    # keep the PE HAM clock-gate open while waiting on tanh
                    for _f in range(2):
                        nc.tensor.matmul(warm_ps[:], lhsT=junk[:, :H], rhs=junk[:],
                                         start=True, stop=True)

            # ---- logits + log_softmax ----
            h0 = h_tiles[0][0]                      # [H, TPC]
            lg_ps = minips.tile([NCLS, TPC], dt.float32, tag="mini", name="lg_ps")
            nc.tensor.matmul(lg_ps[:], lhsT=wc16[:],
                             rhs=h0[:], start=True, stop=True)
            lgT = smpool.tile([NCLS, TPC], dt.float32, tag="lgT")
            nc.vector.tensor_tensor(out=lgT[:], in0=lg_ps[:],
                                    in1=bc_t[:].to_broadcast([NCLS, TPC]), op=ALU.add)
            tr_ps = minips.tile([TPC, NCLS], dt.float32, tag="mini", name="tr_ps")
            nc.tensor.transpose(tr_ps[:], lgT[:], id6[:])
            lg = smpool.tile([TPC, NCLS], dt.float32, tag="lg")
            nc.vector.tensor_copy(lg[:], tr_ps[:])
            ex = smpool.tile([TPC, NCLS], dt.float32, tag="ex")
            nc.scalar.activation(ex[:], lg[:], AF.Exp)
            s = smpool.tile([TPC, 1], dt.float32, tag="s")
            nc.vector.tensor_reduce(s[:], ex[:], axis=AX.X, op=ALU.add)
            lns = smpool.tile([TPC, 1], dt.float32, tag="lns")
            nc.scalar.activation(lns[:], s[:], AF.Ln)
            res = smpool.tile([TPC, NCLS], dt.float32, tag="res")
            nc.vector.tensor_scalar(out=res[:], in0=lg[:], scalar1=lns[:], scalar2=None,
                                    op0=ALU.subtract)
            nc.sync.dma_start(out=out_d[:], in_=res[:])

    nc.compile()
    return nc, "out"


def _get_program():
    global _PROGRAM
    if _PROGRAM is None:
        _PROGRAM = _build_program()
    return _PROGRAM


def _host_inputs(node_type, E, Wh, bh, Wc, bc):
    """Build per-core input maps (host side: sharding + index re-encoding)."""
    nt = np.asarray(node_type).astype(np.int64).reshape(B, M)
    E = np.asarray(E, dtype=np.float32)
    Wh = np.asarray(Wh, dtype=np.float32)
    bh = np.asarray(bh, dtype=np.float32)
    Wc = np.asarray(Wc, dtype=np.float32)
    bc = np.asarray(bc, dtype=np.float32)

    shared = {
        "e_bf": E.astype(np.float16),
        "e_t": np.ascontiguousarray(E.T),
        "wh_b": Wh.astype(np.float16),
        "bh": bh.reshape(H, 1),
        "wc": Wc,
        "bc": bc.reshape(NCLS, 1),
    }

    in_maps = []
    for c in range(CORES):
        ntc = nt[c * TPC:(c + 1) * TPC]                  # [32, 1023]
        ohf = np.zeros((V, TOTAL_COLS), dtype=np.float32)
        # C8: summed one-hots of the two leaf children of each level-8 parent
        lt = ntc[:, 511:1023]                            # [32, 512] leaves
        a = lt[:, 0::2].ravel()
        b = lt[:, 1::2].ravel()
        cols = np.arange(N8)
        np.add.at(ohf, (a, OFF_C8 + cols), 1.0)
        np.add.at(ohf, (b, OFF_C8 + cols), 1.0)
        # per-level x one-hots
        for d in range(8, -1, -1):
            idx = ntc[:, (1 << d) - 1:(2 << d) - 1].ravel()
            ohf[idx, OFF_X[d] + np.arange(idx.size)] = 1.0
        in_maps.append({"oh": ohf.astype(ml_dtypes.float8_e4m3), **shared})
    return in_maps


def kernel(node_type, parent_idx, depth, root_idx, E, Wh, bh, Wc, bc,
           _trace=False, _sim=False):
    from concourse.bass_utils import run_bass_kernel_spmd

    nc, out_name = _get_program()
    in_maps = _host_inputs(node_type, E, Wh, bh, Wc, bc)

    if _sim:
        from concourse.bass_interp import CoreSim
        outs = []
        for m in in_maps[:_sim if isinstance(_sim, int) and _sim > 1 else CORES]:
            sim = CoreSim(nc, trace=False)
            for k, v in m.items():
                sim.tensor(k)[:] = v
            sim.simulate(check_with_hw=False)
            outs.append(np.array(sim.tensor(out_name)))
        return np.concatenate(outs, axis=0).astype(np.float32)

    results = run_bass_kernel_spmd(
        nc, in_maps, core_ids=list(range(CORES)), trace=_trace,
    )
    out = np.concatenate([r[out_name] for r in results.results], axis=0)
    out = np.ascontiguousarray(out).astype(np.float32)
    if _trace:
        return out, results
    return out



# revision 19
# speedup vs baseline: 1.0402x; 1.0095x over previous
"""Trainium2 Bass kernel for CodeRecursiveNeuralNetworks (tree-RNN over complete
binary trees, heap layout).

Math (per tree, heap order: node i has parent (i-1)//2, level d = [2^d-1, 2^{d+1}-1)):
    x = E[node_type];  h_leaf = tanh(x_leaf)
    for d = 8..0:  h_d = tanh(x_d + (h_{d+1,even} + h_{d+1,odd}) @ Wh + bh)
    logits = h_root @ Wc + bc;  out = log_softmax(logits)

Strategy (8 cores, data-parallel over trees; 32 trees/core, no collectives):
  - Everything in "transposed" layout [H=128 partitions, nodes free]; with levels
    stored tree-major, children of parent column p are exactly columns 2p, 2p+1.
  - Embedding lookups become one-hot matmuls on the TensorEngine (VOCAB=100 <= 128):
    the host re-encodes node_type as fp8e4 one-hot columns (index re-encoding only),
    streamed as a few big DMA "bands" over both HWDGE rings.
  - The leaf level is folded away: G = tanh(E) @ Wh is precomputed on device, and
    level-8 gets  psum8 = E^T @ onehot8 + G^T @ C  where C[:,p] = onehot(a_p)+onehot(b_p)
    sums the two leaf children of parent p (so leaves never materialize).
  - Levels 7..6 (big): pairsum on VectorE (stride-2 add, fp16 out) + one fp16 Wh
    matmul. Levels 5..0 (latency-bound): pairsum folded into the PE as two stride-2
    fp16 matmuls, h stored fp16 — no VectorE hop in the serial chain.
  - tanh+bias fused on ScalarE reading PSUM directly; h for levels 8..7 kept fp32.
  - A 26-matmul warm-up burst opens the PE HAM clock gate (1.2 -> 2.4 GHz) while
    the first one-hot bands are still in flight.
  - Final log_softmax on device; per-core output [32, 6] fp32, concatenated on host.
  - Tail levels keep the HAM clock-gate open with junk filler matmuls; matmul
    emission is two-pass per psum tile so accumulating pairs pipeline.
  - Measured: ~47 us HW exec, rel err ~8e-4 vs fp32 reference.
"""

import numpy as np
import ml_dtypes

B = 256
M = 1023
H = 128
V = 100
NCLS = 6
MAX_DEPTH = 10
CORES = 8
TPC = B // CORES          # trees per core
N8 = TPC * 256            # level-8 nodes per core (8192)

# one-hot column block offsets within the single "oh" input tensor
#   [ C8 (8192) | x8 (8192) | x7 (4096) | ... | x0 (32) ]
_LEVEL_N = {d: TPC * (1 << d) for d in range(MAX_DEPTH)}   # cols per level, per core
OFF_C8 = 0
_off = N8
OFF_X = {}
for _d in range(8, -1, -1):
    OFF_X[_d] = _off
    _off += _LEVEL_N[_d]
TOTAL_COLS = _off          # 24544

BF16 = ml_dtypes.bfloat16

_PROGRAM = None            # cached (nc, out_name)


def _build_program():
    import concourse.bacc as bacc
    import concourse.tile as tile
    import concourse.mybir as mybir
    from concourse.masks import make_identity

    dt = mybir.dt
    AF = mybir.ActivationFunctionType
    ALU = mybir.AluOpType
    AX = mybir.AxisListType

    nc = bacc.Bacc("TRN2", target_bir_lowering=False, debug=False)

    oh_d = nc.dram_tensor("oh", [V, TOTAL_COLS], dt.float8e4, kind="ExternalInput")
    ebf_d = nc.dram_tensor("e_bf", [V, H], dt.float16, kind="ExternalInput")
    et_d = nc.dram_tensor("e_t", [H, V], dt.float32, kind="ExternalInput")
    whb_d = nc.dram_tensor("wh_b", [H, H], dt.float16, kind="ExternalInput")
    bh_d = nc.dram_tensor("bh", [H, 1], dt.float32, kind="ExternalInput")
    wc_d = nc.dram_tensor("wc", [H, NCLS], dt.float32, kind="ExternalInput")
    bc_d = nc.dram_tensor("bc", [NCLS, 1], dt.float32, kind="ExternalInput")
    out_d = nc.dram_tensor("out", [TPC, NCLS], dt.float32, kind="ExternalOutput")

    with tile.TileContext(nc) as tc:
        with (
            tc.tile_pool(name="const", bufs=1) as cpool,
            tc.tile_pool(name="bandp", bufs=12) as bandpool,
            tc.tile_pool(name="hp", bufs=1) as hpool,
            tc.tile_pool(name="aggp", bufs=3) as aggpool,
            tc.tile_pool(name="psp", bufs=3, space="PSUM") as pspool,
            tc.tile_pool(name="minips", bufs=2, space="PSUM") as minips,
            tc.tile_pool(name="smallp", bufs=1) as smpool,
        ):
            # ---- PE warm-up first: junk memset is the only dependency ----
            junk = cpool.tile([H, 512], dt.bfloat16, tag="junk")
            nc.gpsimd.memset(junk[:], 0)
            warm_ps = minips.tile([H, 512], dt.float32, tag="mini", name="warm_ps")
            for k in range(30):
                nc.tensor.matmul(warm_ps[:], lhsT=junk[:, :H], rhs=junk[:],
                                 start=True, stop=True)

            # ---- constants ----
            e_bf = cpool.tile([V, H], dt.float16, tag="e_bf")
            nc.gpsimd.dma_start(out=e_bf[:], in_=ebf_d[:])
            et = cpool.tile([H, V], dt.float32, tag="et")
            nc.gpsimd.dma_start(out=et[:], in_=et_d[:])
            whb = cpool.tile([H, H], dt.float16, tag="whb")
            nc.gpsimd.dma_start(out=whb[:], in_=whb_d[:])
            bh_t = cpool.tile([H, 1], dt.float32, tag="bh")
            nc.gpsimd.dma_start(out=bh_t[:], in_=bh_d[:])
            wc_t = cpool.tile([H, NCLS], dt.float32, tag="wc")
            nc.gpsimd.dma_start(out=wc_t[:], in_=wc_d[:])
            bc_t = cpool.tile([NCLS, 1], dt.float32, tag="bc")
            nc.gpsimd.dma_start(out=bc_t[:], in_=bc_d[:])
            id6 = cpool.tile([NCLS, NCLS], dt.float32, tag="id6")
            make_identity(nc, id6[:])
            wc16 = cpool.tile([H, NCLS], dt.float16, tag="wc16")
            nc.vector.tensor_copy(wc16[:], wc_t[:])

            h_tiles = {d: [] for d in range(9)}

            # ---- one-hot bands: few big DMAs instead of many small ones ----
            BAND = 2048
            bands = {}
            _band_eng = [0]

            def load_band(col0, ncols, nm):
                t = bandpool.tile([V, BAND], dt.float8e4, tag="band", name=nm)
                eng = nc.sync if _band_eng[0] % 2 == 0 else nc.scalar
                _band_eng[0] += 1
                eng.dma_start(out=t[:, :ncols], in_=oh_d[:, col0:col0 + ncols])
                return t

            def band_slice(kind, d, gcol, w):
                base = OFF_C8 if kind == "c" else OFF_X[d]
                bi = gcol // BAND
                off = gcol % BAND
                return bands[(kind, d, bi)][:, off:off + w]

            # preload every band up-front (c8/x8 first), alternating rings.
            band_order = [("c", 8), ("x", 8), ("x", 7), ("x", 6), ("x", 5),
                          ("x", 4), ("x", 3), ("x", 2), ("x", 1), ("x", 0)]
            band_reqs = []
            for kind, d in band_order:
                base = OFF_C8 if kind == "c" else OFF_X[d]
                n = N8 if kind == "c" else _LEVEL_N[d]
                for bi in range((n + BAND - 1) // BAND):
                    band_reqs.append((kind, d, bi, base, min(BAND, n - bi * BAND)))
            # first 4 bands now (2 per ring)...
            for kind, d, bi, base, ncols in band_reqs[:4]:
                bands[(kind, d, bi)] = load_band(base + bi * BAND, ncols,
                                                 f"band_{kind}{d}_{bi}")

            # ---- G = tanh(E) @ Wh  (as lhsT [V, H] bf16) ----
            tanh_et = cpool.tile([H, V], dt.float16, tag="tanh_et")
            nc.scalar.activation(tanh_et[:], et[:], AF.Tanh)
            g_ps = minips.tile([V, H], dt.float32, tag="mini")
            nc.tensor.matmul(g_ps[:], lhsT=tanh_et[:], rhs=whb[:], start=True, stop=True)
            g_sb = cpool.tile([V, H], dt.float16, tag="g_sb")
            nc.vector.tensor_copy(g_sb[:], g_ps[:])

            # ...remaining bands after the tanh is queued on ACT
            for kind, d, bi, base, ncols in band_reqs[4:]:
                bands[(kind, d, bi)] = load_band(base + bi * BAND, ncols,
                                                 f"band_{kind}{d}_{bi}")

            # ---- level 8: 2048-col weight-runs (E x4 then G x4) ----
            n = _LEVEL_N[8]
            for blk in range(0, n, 2048):
                pss = []
                for pt in range(blk, blk + 2048, 1024):
                    ps = pspool.tile([H, 1024], dt.float32, tag="ps",
                                     name=f"ps8_{pt}")
                    pss.append((ps, pt))
                for ps, pt in pss:
                    for sub in (0, 512):
                        nc.tensor.matmul(ps[:, sub:sub + 512], lhsT=e_bf[:],
                                         rhs=band_slice("x", 8, pt + sub, 512),
                                         start=True, stop=False)
                for ps, pt in pss:
                    for sub in (0, 512):
                        nc.tensor.matmul(ps[:, sub:sub + 512], lhsT=g_sb[:],
                                         rhs=band_slice("c", 8, pt + sub, 512),
                                         start=False, stop=True)
                for ps, pt in pss:
                    if pt % 2048 == 0:
                        ht = hpool.tile([H, 2048], dt.float32,
                                        tag=f"h8_{pt}", name=f"h8_{pt}")
                        h_tiles[8].append(ht)
                    ht = h_tiles[8][pt // 2048]
                    nc.scalar.activation(ht[:, pt % 2048:pt % 2048 + 1024],
                                         ps[:], AF.Tanh, bias=bh_t[:])

            # ---- levels 7..0 ----
            for d in range(7, -1, -1):
                n = _LEVEL_N[d]
                h_dt = dt.float32 if d >= 7 else dt.float16
                pe_pairsum = d <= 5
                PSF = 1024 if d >= 7 else 512
                for pt in range(0, n, PSF):
                    PT = min(PSF, n - pt)
                    ps = pspool.tile([H, 1024], dt.float32, tag="ps",
                                     name=f"ps{d}_{pt}")
                    if d < 8:
                        hc = h_tiles[d + 1][(2 * pt) // 2048]
                        off = (2 * pt) % 2048
                        pairs = hc[:, off:off + 2 * PT].rearrange(
                            "p (n two) -> p n two", two=2)
                        if not pe_pairsum:
                            agg = aggpool.tile([H, 1024], dt.float16, tag="agg",
                                               name=f"agg{d}_{pt}")
                            nc.vector.tensor_tensor(
                                out=agg[:, :PT].rearrange("p (n one) -> p n one",
                                                          one=1),
                                in0=pairs[:, :, 0:1],
                                in1=pairs[:, :, 1:2],
                                op=ALU.add,
                            )
                    subs = [(ps[:, sub:sub + min(512, PT - sub)], pt + sub, sub,
                             min(512, PT - sub)) for sub in range(0, PT, 512)]
                    # two-pass emission: consecutive matmuls hit different psum
                    # slices so fill/drain overlap (same-slice accumulating
                    # pairs serialize on the PE)
                    for sl, gcol, sub, w in subs:
                        nc.tensor.matmul(sl, lhsT=e_bf[:],
                                         rhs=band_slice("x", d, gcol, w),
                                         start=True, stop=False)
                    if pe_pairsum:
                        for sl, gcol, sub, w in subs:
                            nc.tensor.matmul(sl, lhsT=whb[:],
                                             rhs=pairs[:, sub:sub + w, 0:1],
                                             start=False, stop=False)
                        for sl, gcol, sub, w in subs:
                            nc.tensor.matmul(sl, lhsT=whb[:],
                                             rhs=pairs[:, sub:sub + w, 1:2],
                                             start=False, stop=True)
                    else:
                        for sl, gcol, sub, w in subs:
                            nc.tensor.matmul(sl, lhsT=whb[:],
                                             rhs=agg[:, sub:sub + w],
                                             start=False, stop=True)
                    if pt % 2048 == 0:
                        ht = hpool.tile([H, min(2048, n)], h_dt,
                                        tag=f"h{d}_{pt}", name=f"h{d}_{pt}")
                        h_tiles[d].append(ht)
                    ht = h_tiles[d][pt // 2048]
                    nc.scalar.activation(ht[:, pt % 2048:pt % 2048 + PT],
                                         ps[:, :PT], AF.Tanh, bias=bh_t[:])
                if d <= 5:
                    # keep the PE HAM clock-gate open while waiting on tanh
                    for _f in range(2):
                        nc.tensor.matmul(warm_ps[:], lhsT=junk[:, :H], rhs=junk[:],
                                         start=True, stop=True)

            # ---- logits + log_softmax ----
            h0 = h_tiles[0][0]                      # [H, TPC]
            lg_ps = minips.tile([NCLS, TPC], dt.float32, tag="mini", name="lg_ps")
            nc.tensor.matmul(lg_ps[:], lhsT=wc16[:],
                             rhs=h0[:], start=True, stop=True)
            lgT = smpool.tile([NCLS, TPC], dt.float32, tag="lgT")
            nc.vector.tensor_tensor(out=lgT[:], in0=lg_ps[:],
                                    in1=bc_t[:].to_broadcast([NCLS, TPC]), op=ALU.add)
            tr_ps = minips.tile([TPC, NCLS], dt.float32, tag="mini", name="tr_ps")
            nc.tensor.transpose(tr_ps[:], lgT[:], id6[:])
            lg = smpool.tile([TPC, NCLS], dt.float32, tag="lg")
            nc.vector.tensor_copy(lg[:], tr_ps[:])
            ex = smpool.tile([TPC, NCLS], dt.float32, tag="ex")
            nc.scalar.activation(ex[:], lg[:], AF.Exp)
            s = smpool.tile([TPC, 1], dt.float32, tag="s")
            nc.vector.tensor_reduce(s[:], ex[:], axis=AX.X, op=ALU.add)
            lns = smpool.tile([TPC, 1], dt.float32, tag="lns")
            nc.scalar.activation(lns[:], s[:], AF.Ln)
            res = smpool.tile([TPC, NCLS], dt.float32, tag="res")
            nc.vector.tensor_scalar(out=res[:], in0=lg[:], scalar1=lns[:], scalar2=None,
                                    op0=ALU.subtract)
            nc.sync.dma_start(out=out_d[:], in_=res[:])

    nc.compile()
    return nc, "out"


def _get_program():
    global _PROGRAM
    if _PROGRAM is None:
        _PROGRAM = _build_program()
    return _PROGRAM


def _host_inputs(node_type, E, Wh, bh, Wc, bc):
    """Build per-core input maps (host side: sharding + index re-encoding)."""
    nt = np.asarray(node_type).astype(np.int64).reshape(B, M)
    E = np.asarray(E, dtype=np.float32)
    Wh = np.asarray(Wh, dtype=np.float32)
    bh = np.asarray(bh, dtype=np.float32)
    Wc = np.asarray(Wc, dtype=np.float32)
    bc = np.asarray(bc, dtype=np.float32)

    shared = {
        "e_bf": E.astype(np.float16),
        "e_t": np.ascontiguousarray(E.T),
        "wh_b": Wh.astype(np.float16),
        "bh": bh.reshape(H, 1),
        "wc": Wc,
        "bc": bc.reshape(NCLS, 1),
    }

    in_maps = []
    for c in range(CORES):
        ntc = nt[c * TPC:(c + 1) * TPC]                  # [32, 1023]
        ohf = np.zeros((V, TOTAL_COLS), dtype=np.float32)
        # C8: summed one-hots of the two leaf children of each level-8 parent
        lt = ntc[:, 511:1023]                            # [32, 512] leaves
        a = lt[:, 0::2].ravel()
        b = lt[:, 1::2].ravel()
        cols = np.arange(N8)
        np.add.at(ohf, (a, OFF_C8 + cols), 1.0)
        np.add.at(ohf, (b, OFF_C8 + cols), 1.0)
        # per-level x one-hots
        for d in range(8, -1, -1):
            idx = ntc[:, (1 << d) - 1:(2 << d) - 1].ravel()
            ohf[idx, OFF_X[d] + np.arange(idx.size)] = 1.0
        in_maps.append({"oh": ohf.astype(ml_dtypes.float8_e4m3), **shared})
    return in_maps


def kernel(node_type, parent_idx, depth, root_idx, E, Wh, bh, Wc, bc,
           _trace=False, _sim=False):
    from concourse.bass_utils import run_bass_kernel_spmd

    nc, out_name = _get_program()
    in_maps = _host_inputs(node_type, E, Wh, bh, Wc, bc)

    if _sim:
        from concourse.bass_interp import CoreSim
        outs = []
        for m in in_maps[:_sim if isinstance(_sim, int) and _sim > 1 else CORES]:
            sim = CoreSim(nc, trace=False)
            for k, v in m.items():
                sim.tensor(k)[:] = v
            sim.simulate(check_with_hw=False)
            outs.append(np.array(sim.tensor(out_name)))
        return np.concatenate(outs, axis=0).astype(np.float32)

    results = run_bass_kernel_spmd(
        nc, in_maps, core_ids=list(range(CORES)), trace=_trace,
    )
    out = np.concatenate([r[out_name] for r in results.results], axis=0)
    out = np.ascontiguousarray(out).astype(np.float32)
    if _trace:
        return out, results
    return out



# revision 20
# speedup vs baseline: 1.0405x; 1.0003x over previous
"""Trainium2 Bass kernel for CodeRecursiveNeuralNetworks (tree-RNN over complete
binary trees, heap layout).

Math (per tree, heap order: node i has parent (i-1)//2, level d = [2^d-1, 2^{d+1}-1)):
    x = E[node_type];  h_leaf = tanh(x_leaf)
    for d = 8..0:  h_d = tanh(x_d + (h_{d+1,even} + h_{d+1,odd}) @ Wh + bh)
    logits = h_root @ Wc + bc;  out = log_softmax(logits)

Strategy (8 cores, data-parallel over trees; 32 trees/core, no collectives):
  - Everything in "transposed" layout [H=128 partitions, nodes free]; with levels
    stored tree-major, children of parent column p are exactly columns 2p, 2p+1.
  - Embedding lookups become one-hot matmuls on the TensorEngine (VOCAB=100 <= 128):
    the host re-encodes node_type as fp8e4 one-hot columns (index re-encoding only),
    streamed as a few big DMA "bands" over both HWDGE rings.
  - The leaf level is folded away: G = tanh(E) @ Wh is precomputed on device, and
    level-8 gets  psum8 = E^T @ onehot8 + G^T @ C  where C[:,p] = onehot(a_p)+onehot(b_p)
    sums the two leaf children of parent p (so leaves never materialize).
  - Levels 7..6 (big): pairsum on VectorE (stride-2 add, fp16 out) + one fp16 Wh
    matmul. Levels 5..0 (latency-bound): pairsum folded into the PE as two stride-2
    fp16 matmuls, h stored fp16 — no VectorE hop in the serial chain.
  - tanh+bias fused on ScalarE reading PSUM directly; h for levels 8..7 kept fp32.
  - A 26-matmul warm-up burst opens the PE HAM clock gate (1.2 -> 2.4 GHz) while
    the first one-hot bands are still in flight.
  - Final log_softmax on device; per-core output [32, 6] fp32, concatenated on host.
  - Tail levels keep the HAM clock-gate open with junk filler matmuls; matmul
    emission is two-pass per psum tile so accumulating pairs pipeline.
  - Measured: ~47 us HW exec, rel err ~8e-4 vs fp32 reference.
"""

import numpy as np
import ml_dtypes

B = 256
M = 1023
H = 128
V = 100
NCLS = 6
MAX_DEPTH = 10
CORES = 8
TPC = B // CORES          # trees per core
N8 = TPC * 256            # level-8 nodes per core (8192)

# one-hot column block offsets within the single "oh" input tensor
#   [ C8 (8192) | x8 (8192) | x7 (4096) | ... | x0 (32) ]
_LEVEL_N = {d: TPC * (1 << d) for d in range(MAX_DEPTH)}   # cols per level, per core
OFF_C8 = 0
_off = N8
OFF_X = {}
for _d in range(8, -1, -1):
    OFF_X[_d] = _off
    _off += _LEVEL_N[_d]
TOTAL_COLS = _off          # 24544

BF16 = ml_dtypes.bfloat16

_PROGRAM = None            # cached (nc, out_name)


def _build_program():
    import concourse.bacc as bacc
    import concourse.tile as tile
    import concourse.mybir as mybir
    from concourse.masks import make_identity

    dt = mybir.dt
    AF = mybir.ActivationFunctionType
    ALU = mybir.AluOpType
    AX = mybir.AxisListType

    nc = bacc.Bacc("TRN2", target_bir_lowering=False, debug=False)

    oh_d = nc.dram_tensor("oh", [V, TOTAL_COLS], dt.float8e4, kind="ExternalInput")
    ebf_d = nc.dram_tensor("e_bf", [V, H], dt.float16, kind="ExternalInput")
    et_d = nc.dram_tensor("e_t", [H, V], dt.float32, kind="ExternalInput")
    whb_d = nc.dram_tensor("wh_b", [H, H], dt.float16, kind="ExternalInput")
    bh_d = nc.dram_tensor("bh", [H, 1], dt.float32, kind="ExternalInput")
    wc_d = nc.dram_tensor("wc", [H, NCLS], dt.float32, kind="ExternalInput")
    bc_d = nc.dram_tensor("bc", [NCLS, 1], dt.float32, kind="ExternalInput")
    out_d = nc.dram_tensor("out", [TPC, NCLS], dt.float32, kind="ExternalOutput")

    with tile.TileContext(nc) as tc:
        with (
            tc.tile_pool(name="const", bufs=1) as cpool,
            tc.tile_pool(name="bandp", bufs=12) as bandpool,
            tc.tile_pool(name="hp", bufs=1) as hpool,
            tc.tile_pool(name="aggp", bufs=3) as aggpool,
            tc.tile_pool(name="psp", bufs=3, space="PSUM") as pspool,
            tc.tile_pool(name="minips", bufs=2, space="PSUM") as minips,
            tc.tile_pool(name="smallp", bufs=1) as smpool,
        ):
            # ---- PE warm-up first: junk memset is the only dependency ----
            junk = cpool.tile([H, 512], dt.bfloat16, tag="junk")
            nc.gpsimd.memset(junk[:], 0)
            warm_ps = minips.tile([H, 512], dt.float32, tag="mini", name="warm_ps")
            for k in range(30):
                nc.tensor.matmul(warm_ps[:], lhsT=junk[:, :H], rhs=junk[:],
                                 start=True, stop=True)

            # ---- constants ----
            e_bf = cpool.tile([V, H], dt.float16, tag="e_bf")
            nc.gpsimd.dma_start(out=e_bf[:], in_=ebf_d[:])
            et = cpool.tile([H, V], dt.float32, tag="et")
            nc.gpsimd.dma_start(out=et[:], in_=et_d[:])
            whb = cpool.tile([H, H], dt.float16, tag="whb")
            nc.gpsimd.dma_start(out=whb[:], in_=whb_d[:])
            bh_t = cpool.tile([H, 1], dt.float32, tag="bh")
            nc.gpsimd.dma_start(out=bh_t[:], in_=bh_d[:])
            wc_t = cpool.tile([H, NCLS], dt.float32, tag="wc")
            nc.gpsimd.dma_start(out=wc_t[:], in_=wc_d[:])
            bc_t = cpool.tile([NCLS, 1], dt.float32, tag="bc")
            nc.gpsimd.dma_start(out=bc_t[:], in_=bc_d[:])
            id6 = cpool.tile([NCLS, NCLS], dt.float32, tag="id6")
            make_identity(nc, id6[:])
            wc16 = cpool.tile([H, NCLS], dt.float16, tag="wc16")
            nc.vector.tensor_copy(wc16[:], wc_t[:])

            h_tiles = {d: [] for d in range(9)}

            # ---- one-hot bands: few big DMAs instead of many small ones ----
            BAND = 2048
            bands = {}
            _band_eng = [0]

            def load_band(col0, ncols, nm):
                t = bandpool.tile([V, BAND], dt.float8e4, tag="band", name=nm)
                eng = nc.sync if _band_eng[0] % 2 == 0 else nc.scalar
                _band_eng[0] += 1
                eng.dma_start(out=t[:, :ncols], in_=oh_d[:, col0:col0 + ncols])
                return t

            def band_slice(kind, d, gcol, w):
                base = OFF_C8 if kind == "c" else OFF_X[d]
                bi = gcol // BAND
                off = gcol % BAND
                return bands[(kind, d, bi)][:, off:off + w]

            # preload every band up-front (c8/x8 first), alternating rings.
            band_order = [("c", 8), ("x", 8), ("x", 7), ("x", 6), ("x", 5),
                          ("x", 4), ("x", 3), ("x", 2), ("x", 1), ("x", 0)]
            band_reqs = []
            for kind, d in band_order:
                base = OFF_C8 if kind == "c" else OFF_X[d]
                n = N8 if kind == "c" else _LEVEL_N[d]
                for bi in range((n + BAND - 1) // BAND):
                    band_reqs.append((kind, d, bi, base, min(BAND, n - bi * BAND)))
            # first 4 bands now (2 per ring)...
            for kind, d, bi, base, ncols in band_reqs[:4]:
                bands[(kind, d, bi)] = load_band(base + bi * BAND, ncols,
                                                 f"band_{kind}{d}_{bi}")

            # ---- G = tanh(E) @ Wh  (as lhsT [V, H] bf16) ----
            tanh_et = cpool.tile([H, V], dt.float16, tag="tanh_et")
            nc.scalar.activation(tanh_et[:], et[:], AF.Tanh)
            g_ps = minips.tile([V, H], dt.float32, tag="mini")
            nc.tensor.matmul(g_ps[:], lhsT=tanh_et[:], rhs=whb[:], start=True, stop=True)
            g_sb = cpool.tile([V, H], dt.float16, tag="g_sb")
            nc.vector.tensor_copy(g_sb[:], g_ps[:])

            # ...remaining bands after the tanh is queued on ACT
            for kind, d, bi, base, ncols in band_reqs[4:]:
                bands[(kind, d, bi)] = load_band(base + bi * BAND, ncols,
                                                 f"band_{kind}{d}_{bi}")

            # ---- level 8: 2048-col weight-runs (E x4 then G x4) ----
            n = _LEVEL_N[8]
            for blk in range(0, n, 2048):
                pss = []
                for pt in range(blk, blk + 2048, 1024):
                    ps = pspool.tile([H, 1024], dt.float32, tag="ps",
                                     name=f"ps8_{pt}")
                    pss.append((ps, pt))
                for ps, pt in pss:
                    for sub in (0, 512):
                        nc.tensor.matmul(ps[:, sub:sub + 512], lhsT=e_bf[:],
                                         rhs=band_slice("x", 8, pt + sub, 512),
                                         start=True, stop=False)
                for ps, pt in pss:
                    for sub in (0, 512):
                        nc.tensor.matmul(ps[:, sub:sub + 512], lhsT=g_sb[:],
                                         rhs=band_slice("c", 8, pt + sub, 512),
                                         start=False, stop=True)
                for ps, pt in pss:
                    if pt % 2048 == 0:
                        ht = hpool.tile([H, 2048], dt.float32,
                                        tag=f"h8_{pt}", name=f"h8_{pt}")
                        h_tiles[8].append(ht)
                    ht = h_tiles[8][pt // 2048]
                    nc.scalar.activation(ht[:, pt % 2048:pt % 2048 + 1024],
                                         ps[:], AF.Tanh, bias=bh_t[:])

            # ---- level 7: same 2048-col weight-run grouping ----
            n = _LEVEL_N[7]
            for blk in range(0, n, 2048):
                pss = []
                for pt in (blk, blk + 1024):
                    ps = pspool.tile([H, 1024], dt.float32, tag="ps",
                                     name=f"ps7_{pt}")
                    hc = h_tiles[8][(2 * pt) // 2048]
                    off = (2 * pt) % 2048
                    pairs = hc[:, off:off + 2048].rearrange(
                        "p (n two) -> p n two", two=2)
                    agg = aggpool.tile([H, 1024], dt.float16, tag="agg",
                                       name=f"agg7_{pt}")
                    nc.vector.tensor_tensor(
                        out=agg[:].rearrange("p (n one) -> p n one", one=1),
                        in0=pairs[:, :, 0:1], in1=pairs[:, :, 1:2], op=ALU.add)
                    pss.append((ps, pt, agg))
                for ps, pt, agg in pss:
                    for sub in (0, 512):
                        nc.tensor.matmul(ps[:, sub:sub + 512], lhsT=e_bf[:],
                                         rhs=band_slice("x", 7, pt + sub, 512),
                                         start=True, stop=False)
                for ps, pt, agg in pss:
                    for sub in (0, 512):
                        nc.tensor.matmul(ps[:, sub:sub + 512], lhsT=whb[:],
                                         rhs=agg[:, sub:sub + 512],
                                         start=False, stop=True)
                ht = hpool.tile([H, 2048], dt.float32, tag=f"h7_{blk}",
                                name=f"h7_{blk}")
                h_tiles[7].append(ht)
                for ps, pt, agg in pss:
                    nc.scalar.activation(ht[:, pt % 2048:pt % 2048 + 1024],
                                         ps[:], AF.Tanh, bias=bh_t[:])

            # ---- levels 6..0 ----
            for d in range(6, -1, -1):
                n = _LEVEL_N[d]
                h_dt = dt.float16
                pe_pairsum = d <= 5
                PSF = 512
                for pt in range(0, n, PSF):
                    PT = min(PSF, n - pt)
                    ps = pspool.tile([H, 1024], dt.float32, tag="ps",
                                     name=f"ps{d}_{pt}")
                    if d < 8:
                        hc = h_tiles[d + 1][(2 * pt) // 2048]
                        off = (2 * pt) % 2048
                        pairs = hc[:, off:off + 2 * PT].rearrange(
                            "p (n two) -> p n two", two=2)
                        if not pe_pairsum:
                            agg = aggpool.tile([H, 1024], dt.float16, tag="agg",
                                               name=f"agg{d}_{pt}")
                            nc.vector.tensor_tensor(
                                out=agg[:, :PT].rearrange("p (n one) -> p n one",
                                                          one=1),
                                in0=pairs[:, :, 0:1],
                                in1=pairs[:, :, 1:2],
                                op=ALU.add,
                            )
                    subs = [(ps[:, sub:sub + min(512, PT - sub)], pt + sub, sub,
                             min(512, PT - sub)) for sub in range(0, PT, 512)]
                    # two-pass emission: consecutive matmuls hit different psum
                    # slices so fill/drain overlap (same-slice accumulating
                    # pairs serialize on the PE)
                    for sl, gcol, sub, w in subs:
                        nc.tensor.matmul(sl, lhsT=e_bf[:],
                                         rhs=band_slice("x", d, gcol, w),
                                         start=True, stop=False)
                    if pe_pairsum:
                        for sl, gcol, sub, w in subs:
                            nc.tensor.matmul(sl, lhsT=whb[:],
                                             rhs=pairs[:, sub:sub + w, 0:1],
                                             start=False, stop=False)
                        for sl, gcol, sub, w in subs:
                            nc.tensor.matmul(sl, lhsT=whb[:],
                                             rhs=pairs[:, sub:sub + w, 1:2],
                                             start=False, stop=True)
                    else:
                        for sl, gcol, sub, w in subs:
                            nc.tensor.matmul(sl, lhsT=whb[:],
                                             rhs=agg[:, sub:sub + w],
                                             start=False, stop=True)
                    if pt % 2048 == 0:
                        ht = hpool.tile([H, min(2048, n)], h_dt,
                                        tag=f"h{d}_{pt}", name=f"h{d}_{pt}")
                        h_tiles[d].append(ht)
                    ht = h_tiles[d][pt // 2048]
                    nc.scalar.activation(ht[:, pt % 2048:pt % 2048 + PT],
                                         ps[:, :PT], AF.Tanh, bias=bh_t[:])
                if d <= 5:
                    # keep the PE HAM clock-gate open while waiting on tanh
                    for _f in range(2):
                        nc.tensor.matmul(warm_ps[:], lhsT=junk[:, :H], rhs=junk[:],
                                         start=True, stop=True)

            # ---- logits + log_softmax ----
            h0 = h_tiles[0][0]                      # [H, TPC]
            lg_ps = minips.tile([NCLS, TPC], dt.float32, tag="mini", name="lg_ps")
            nc.tensor.matmul(lg_ps[:], lhsT=wc16[:],
                             rhs=h0[:], start=True, stop=True)
            lgT = smpool.tile([NCLS, TPC], dt.float32, tag="lgT")
            nc.vector.tensor_tensor(out=lgT[:], in0=lg_ps[:],
                                    in1=bc_t[:].to_broadcast([NCLS, TPC]), op=ALU.add)
            tr_ps = minips.tile([TPC, NCLS], dt.float32, tag="mini", name="tr_ps")
            nc.tensor.transpose(tr_ps[:], lgT[:], id6[:])
            lg = smpool.tile([TPC, NCLS], dt.float32, tag="lg")
            nc.vector.tensor_copy(lg[:], tr_ps[:])
            ex = smpool.tile([TPC, NCLS], dt.float32, tag="ex")
            nc.scalar.activation(ex[:], lg[:], AF.Exp)
            s = smpool.tile([TPC, 1], dt.float32, tag="s")
            nc.vector.tensor_reduce(s[:], ex[:], axis=AX.X, op=ALU.add)
            lns = smpool.tile([TPC, 1], dt.float32, tag="lns")
            nc.scalar.activation(lns[:], s[:], AF.Ln)
            res = smpool.tile([TPC, NCLS], dt.float32, tag="res")
            nc.vector.tensor_scalar(out=res[:], in0=lg[:], scalar1=lns[:], scalar2=None,
                                    op0=ALU.subtract)
            nc.sync.dma_start(out=out_d[:], in_=res[:])

    nc.compile()
    return nc, "out"


def _get_program():
    global _PROGRAM
    if _PROGRAM is None:
        _PROGRAM = _build_program()
    return _PROGRAM


def _host_inputs(node_type, E, Wh, bh, Wc, bc):
    """Build per-core input maps (host side: sharding + index re-encoding)."""
    nt = np.asarray(node_type).astype(np.int64).reshape(B, M)
    E = np.asarray(E, dtype=np.float32)
    Wh = np.asarray(Wh, dtype=np.float32)
    bh = np.asarray(bh, dtype=np.float32)
    Wc = np.asarray(Wc, dtype=np.float32)
    bc = np.asarray(bc, dtype=np.float32)

    shared = {
        "e_bf": E.astype(np.float16),
        "e_t": np.ascontiguousarray(E.T),
        "wh_b": Wh.astype(np.float16),
        "bh": bh.reshape(H, 1),
        "wc": Wc,
        "bc": bc.reshape(NCLS, 1),
    }

    in_maps = []
    for c in range(CORES):
        ntc = nt[c * TPC:(c + 1) * TPC]                  # [32, 1023]
        ohf = np.zeros((V, TOTAL_COLS), dtype=np.float32)
        # C8: summed one-hots of the two leaf children of each level-8 parent
        lt = ntc[:, 511:1023]                            # [32, 512] leaves
        a = lt[:, 0::2].ravel()
        b = lt[:, 1::2].ravel()
        cols = np.arange(N8)
        np.add.at(ohf, (a, OFF_C8 + cols), 1.0)
        np.add.at(ohf, (b, OFF_C8 + cols), 1.0)
        # per-level x one-hots
        for d in range(8, -1, -1):
            idx = ntc[:, (1 << d) - 1:(2 << d) - 1].ravel()
            ohf[idx, OFF_X[d] + np.arange(idx.size)] = 1.0
        in_maps.append({"oh": ohf.astype(ml_dtypes.float8_e4m3), **shared})
    return in_maps


def kernel(node_type, parent_idx, depth, root_idx, E, Wh, bh, Wc, bc,
           _trace=False, _sim=False):
    from concourse.bass_utils import run_bass_kernel_spmd

    nc, out_name = _get_program()
    in_maps = _host_inputs(node_type, E, Wh, bh, Wc, bc)

    if _sim:
        from concourse.bass_interp import CoreSim
        outs = []
        for m in in_maps[:_sim if isinstance(_sim, int) and _sim > 1 else CORES]:
            sim = CoreSim(nc, trace=False)
            for k, v in m.items():
                sim.tensor(k)[:] = v
            sim.simulate(check_with_hw=False)
            outs.append(np.array(sim.tensor(out_name)))
        return np.concatenate(outs, axis=0).astype(np.float32)

    results = run_bass_kernel_spmd(
        nc, in_maps, core_ids=list(range(CORES)), trace=_trace,
    )
    out = np.concatenate([r[out_name] for r in results.results], axis=0)
    out = np.ascontiguousarray(out).astype(np.float32)
    if _trace:
        return out, results
    return out

